# revision 1
# baseline (speedup 1.0000x reference)
"""Multi-head causal attention (B=2, S=2048, D=1024, H=16) on 8 TRN2 cores.

Sharding: core = (batch, group-of-4-heads). Each core computes attention for
its 4 heads of its batch and a rank-256 partial of the output projection;
the host sums the 4 partials per batch. The interleaved head split of the
reference (head h = columns h::16) is undone on the host by permuting the
weight matrices, so on-chip everything is head-contiguous.

On-chip layout (per core, all matmuls in fp32r):
  QT/KT [128, 2048]   head-pair-stacked transposed Q/K (pair p, heads A/B on
                      partitions 0:64 / 64:128)
  S^T   [128, 2, 512] scores for a (k-tile, q-chunk), both heads, one 2-bank
                      PSUM tile; K=64 matmuls row-packed in the PE array
  exp   one ScalarE activation per k-tile over both heads' scores
  PV    oAB[65, 2, 512] += Vaug^T @ P^T; Vaug carries a ones column so row 64
                      accumulates the softmax denominator l
  norm  1/l broadcast via DRAM-roundtrip DMA (stride-0 partition reads are
                      only legal from DRAM), A^T scaled on VectorE
  out   partial = A^T.T @ ow, accumulated over the 2 pairs in PSUM
"""
import sys
sys.path.insert(0, '/opt/trn_rl_repo')

import numpy as np

DIM = 1024
HEADS = 16
S = 2048
B = 2
HD = 64
N_CORES = 8
HPC = 4          # heads per core
PAIRS = 2        # processed as 2 pairs of heads (pair packs the 128-wide PE)
QCH = 512        # q chunk
NKT = S // 128   # k tiles per sequence

_nc_cache = None


def _build(debug=False):
    import concourse.bass as bass
    import concourse.tile as tile
    import concourse.mybir as mybir
    from concourse import bacc
    from concourse.masks import make_identity
    from contextlib import ExitStack

    f32 = mybir.dt.float32
    f32r = mybir.dt.float32r
    Exp = mybir.ActivationFunctionType.Exp

    def bc(ap, n):
        # stride-0 partition broadcast of a [1, ...] DRAM AP to n partitions
        return bass.AP(tensor=ap.tensor, offset=ap.offset,
                       ap=[[0, n]] + [list(d) for d in ap.ap[1:]])

    nc = bacc.Bacc("TRN2", target_bir_lowering=False, debug=False,
                   enable_asserts=False, num_devices=N_CORES)

    xT = nc.dram_tensor("xT", [DIM, S], f32r, kind="ExternalInput").ap()
    qw = nc.dram_tensor("qw", [DIM, 256], f32r, kind="ExternalInput").ap()
    kw = nc.dram_tensor("kw", [DIM, 256], f32r, kind="ExternalInput").ap()
    vw = nc.dram_tensor("vw", [DIM, 256], f32r, kind="ExternalInput").ap()
    ow = nc.dram_tensor("ow", [256, DIM], f32r, kind="ExternalInput").ap()
    masks = nc.dram_tensor("masks", [4, 128, QCH], f32, kind="ExternalInput").ap()
    out = nc.dram_tensor("out", [S, DIM], f32, kind="ExternalOutput").ap()
    if debug:
        dbg = {}
        for name, shape, dt_ in (
                ("dQT0", [128, S], f32r), ("dKT0", [128, S], f32r),
                ("dVT0", [128, S], f32), ("dVaug0", [128, NKT, HD + 1], f32r),
                ("dAT0", [128, S], f32r)):
            dbg[name] = nc.dram_tensor(name, shape, dt_, kind="ExternalOutput").ap()

    with tile.TileContext(nc) as tc, ExitStack() as ctx:
        const_pool = ctx.enter_context(tc.tile_pool(name="const", bufs=1))
        xin_pool = ctx.enter_context(tc.tile_pool(name="xin", bufs=2))
        big_pool = ctx.enter_context(tc.tile_pool(name="big", bufs=1))
        pt_pool = ctx.enter_context(tc.tile_pool(name="pt", bufs=4))
        small_pool = ctx.enter_context(tc.tile_pool(name="small", bufs=4))
        outst_pool = ctx.enter_context(tc.tile_pool(name="outst", bufs=3))
        dram_pool = ctx.enter_context(tc.tile_pool(name="dram", bufs=2, space="DRAM"))
        psum_s = ctx.enter_context(tc.tile_pool(name="psum_s", bufs=2, space="PSUM"))
        psum_o = ctx.enter_context(tc.tile_pool(name="psum_o", bufs=2, space="PSUM"))

        # constants
        qw_sb = const_pool.tile([128, 8, 256], f32r, tag="qw")
        kw_sb = const_pool.tile([128, 8, 256], f32r, tag="kw")
        vw_sb = const_pool.tile([128, 8, 256], f32r, tag="vw")
        nc.sync.dma_start(out=qw_sb, in_=qw.rearrange("(kt p) m -> p kt m", p=128))
        nc.sync.dma_start(out=kw_sb, in_=kw.rearrange("(kt p) m -> p kt m", p=128))
        nc.sync.dma_start(out=vw_sb, in_=vw.rearrange("(kt p) m -> p kt m", p=128))
        ow_sb = const_pool.tile([128, 2, DIM], f32r, tag="ow")
        nc.sync.dma_start(out=ow_sb, in_=ow.rearrange("(t p) n -> p t n", p=128))
        masks_sb = const_pool.tile([128, 4, QCH], f32, tag="masks")
        nc.sync.dma_start(out=masks_sb, in_=masks.rearrange("i p q -> p i q"))
        ident = const_pool.tile([128, 128], f32, tag="ident")
        make_identity(nc, ident)

        QT = [big_pool.tile([128, S], f32r, tag=f"QT{p}", name=f"QT{p}")
              for p in range(PAIRS)]
        KT = [big_pool.tile([128, S], f32r, tag=f"KT{p}", name=f"KT{p}")
              for p in range(PAIRS)]
        VT = [big_pool.tile([128, S], f32, tag=f"VT{p}", name=f"VT{p}")
              for p in range(PAIRS)]
        AT = [big_pool.tile([128, S], f32r, tag=f"AT{p}", name=f"AT{p}")
              for p in range(PAIRS)]
        Vaug = [big_pool.tile([128, NKT, HD + 1], f32r, tag=f"Vaug{h}",
                              name=f"Vaug{h}") for h in range(HPC)]
        ones_c = const_pool.tile([128, NKT, 1], f32, tag="ones")
        nc.vector.memset(ones_c, 1.0)
        for h in range(HPC):
            nc.vector.tensor_copy(out=Vaug[h][:, :, HD:HD + 1], in_=ones_c)

        # ---- Projection / transpose pieces (fed into the attention loop so
        # ---- ScalarE exp work overlaps projection PE work) ----
        xts = {}

        def xt_dma(j):
            qsl = slice(j * QCH, (j + 1) * QCH)
            xt = xin_pool.tile([128, 8, QCH], f32r, tag="xt", name="xt")
            nc.sync.dma_start(
                out=xt,
                in_=xT.rearrange("(kt p) n -> p kt n", p=128)[:, :, qsl])
            xts[j] = xt

        def proj_group(wsb, dstT, j):
            qsl = slice(j * QCH, (j + 1) * QCH)
            ps2 = psum_s.tile([128, 2, QCH], f32, tag="s2", name="ps2")
            for ct in range(PAIRS):
                for kt in range(8):
                    nc.tensor.matmul(
                        ps2[:, ct, :],
                        wsb[:, kt, ct * 128:(ct + 1) * 128],
                        xts[j][:, kt, :],
                        start=(kt == 0), stop=(kt == 7))
            for ct in range(PAIRS):
                nc.vector.tensor_copy(out=dstT[ct][:, qsl], in_=ps2[:, ct, :])

        def transpose_piece(pr, kt):
            pst = psum_s.tile([128, 128], f32, tag="s2", name="pst")
            nc.tensor.transpose(pst, VT[pr][:, kt * 128:(kt + 1) * 128], ident)
            for hh in range(2):
                nc.vector.tensor_copy(
                    out=Vaug[2 * pr + hh][:, kt, 0:HD],
                    in_=pst[:, hh * 64:(hh + 1) * 64])

        def round_feed(j):
            # work pieces producing round-j inputs (proj chunk j + its V
            # transposes), to be sprinkled into round j-1's attention
            items = [lambda: xt_dma(j)]
            for wsb, dstT in ((qw_sb, QT), (kw_sb, KT), (vw_sb, VT)):
                items.append(lambda w=wsb, d=dstT: proj_group(w, d, j))
            for pr in range(PAIRS):
                for ktl in range(4):
                    items.append(
                        lambda q=pr, k=4 * j + ktl: transpose_piece(q, k))
            return items

        # prologue: round 0 inputs emitted directly
        for piece in round_feed(0):
            piece()

        # ---- Phase 2: causal attention, pair-packed; then output proj ----
        # Finalize (recip + normalize + out-proj) for chunk i is EMITTED after
        # chunk i+1's first matmuls so the slow DVE reciprocal never
        # head-of-line blocks the PE instruction stream.
        def finalize_norm(p, j, oAB):
            # l rows out via ScalarE (keeps slow ops off the DVE mask path),
            # broadcast raw l through DRAM, one [128,512] reciprocal, scale.
            qsl = slice(j * QCH, (j + 1) * QCH)
            lsb = small_pool.tile([33, QCH], f32, tag="r", name="lsb")
            nc.scalar.copy(out=lsb[0:1, :], in_=oAB[HD:HD + 1, 0, :])
            nc.scalar.copy(out=lsb[32:33, :], in_=oAB[HD:HD + 1, 1, :])
            rd = dram_pool.tile([33, QCH], f32, tag="rd", name="rd")
            nc.sync.dma_start(out=rd, in_=lsb)
            Rsb = small_pool.tile([128, QCH], f32, tag="Rsb", name="Rsb")
            nc.sync.dma_start(out=Rsb[0:64, :], in_=bc(rd[0:1, :], 64))
            nc.sync.dma_start(out=Rsb[64:128, :], in_=bc(rd[32:33, :], 64))
            rscr = small_pool.tile([128, QCH], f32, tag="rscr", name="rscr")
            with nc.allow_low_precision(reason="recip"):
                nc.vector.reciprocal_approx_accurate(out=Rsb, in_=Rsb,
                                                     scratch=rscr)
            nc.vector.tensor_copy(out=AT[p][0:64, qsl], in_=oAB[0:HD, 0, :])
            nc.vector.tensor_copy(out=AT[p][64:128, qsl], in_=oAB[0:HD, 1, :])
            nc.vector.tensor_mul(AT[p][:, qsl], AT[p][:, qsl], Rsb)

        def emit_outproj(j):
            # output projection for q rows of chunk j (needs both pairs' AT)
            for rt in range(QCH // 128):
                rsl = slice(j * QCH + rt * 128, j * QCH + (rt + 1) * 128)
                po2 = psum_s.tile([128, 2, 512], f32, tag="s2", name="po2")
                for nch in range(DIM // 512):
                    for pp in range(PAIRS):
                        nc.tensor.matmul(
                            po2[:, nch, :], AT[pp][:, rsl],
                            ow_sb[:, pp, nch * 512:(nch + 1) * 512],
                            start=(pp == 0), stop=(pp == PAIRS - 1))
                ot = outst_pool.tile([128, 2, 512], f32, tag="ot", name="ot")
                if rt % 2 == 0:
                    nc.vector.tensor_copy(out=ot, in_=po2)
                else:
                    nc.scalar.copy(out=ot, in_=po2)
                nc.sync.dma_start(
                    out=out[rsl, :].rearrange("p (a b) -> p a b", a=2),
                    in_=ot)

        pending_norm = None
        pending_proj = None
        feed = []
        for j in range(S // QCH):
            if j + 1 < S // QCH:
                feed = round_feed(j + 1)
            for p in range(PAIRS):
                nkt = 4 * (j + 1)
                qsl = slice(j * QCH, (j + 1) * QCH)
                # norm of the previous chunk is pure DVE/ACT/DMA work; emit it
                # before this chunk's matmuls UNLESS this chunk's causal masks
                # start immediately (j == 0), where the recip would delay them.
                if pending_norm is not None and j != 0:
                    finalize_norm(*pending_norm)
                    pending_norm = None
                all_diag = (j == 0)
                oAB = psum_o.tile([HD + 1, 2, QCH], f32, tag="o", name="oAB")
                # 1-deep software pipeline: PV(kt) is emitted after S(kt+1) so
                # the PE never sits at a PV waiting for its exp.
                pv_prev = None
                for kt in range(nkt):
                    ksl = slice(kt * 128, (kt + 1) * 128)
                    sAB = psum_s.tile([128, 2, QCH], f32, tag="s2", name="sAB")
                    for hh in range(2):
                        nc.tensor.matmul(sAB[:, hh, :],
                                         KT[p][hh * 64:(hh + 1) * 64, ksl],
                                         QT[p][hh * 64:(hh + 1) * 64, qsl],
                                         start=True, stop=True)
                    pAB = pt_pool.tile([128, 2, QCH], f32r, tag="pt", name="pAB")
                    nc.scalar.activation(out=pAB, in_=sAB, func=Exp)
                    di = kt - 4 * j
                    if di >= 0:  # diagonal tile: apply causal mask
                        m2 = masks_sb[:, di, :]
                        m2 = bass.AP(tensor=m2.tensor, offset=m2.offset,
                                     ap=[list(m2.ap[0]), [0, 2],
                                         list(m2.ap[1])])
                        nc.vector.tensor_mul(pAB, pAB, m2)
                    if pv_prev is not None:
                        kprev, pprev = pv_prev
                        for hh in range(2):
                            nc.tensor.matmul(oAB[:, hh, :],
                                             Vaug[2 * p + hh][:, kprev, :],
                                             pprev[:, hh, :],
                                             start=(kprev == 0), stop=False)
                    pv_prev = (kt, pAB)
                    if feed:
                        feed.pop(0)()
                    if (pending_norm is not None and not all_diag
                            and kt == min(3, nkt - 1)):
                        finalize_norm(*pending_norm)
                        pending_norm = None
                    if pending_proj is not None and kt == min(9, nkt - 1):
                        emit_outproj(pending_proj)
                        pending_proj = None
                kprev, pprev = pv_prev
                for hh in range(2):
                    nc.tensor.matmul(oAB[:, hh, :],
                                     Vaug[2 * p + hh][:, kprev, :],
                                     pprev[:, hh, :],
                                     start=(kprev == 0), stop=True)
                if pending_norm is not None:
                    # all-diagonal chunk: its masks are done now, safe to emit
                    finalize_norm(*pending_norm)
                pending_norm = (p, j, oAB)
                if p == PAIRS - 1:
                    pending_proj = j
            while feed:
                feed.pop(0)()
        finalize_norm(*pending_norm)
        if pending_proj is not None:
            emit_outproj(pending_proj)

        if debug:
            nc.sync.dma_start(out=dbg["dAT0"], in_=AT[0])

    nc.compile()
    return nc


def _get_nc():
    global _nc_cache
    if _nc_cache is None:
        _nc_cache = _build()
    return _nc_cache


def _prep_inputs(x, qw, kw, vw, ow):
    # undo interleaved head split: head h = cols h::16 -> contiguous blocks
    perm = np.concatenate([np.arange(h, DIM, HEADS) for h in range(HEADS)])
    qw_p = (qw[:, perm] / np.float32(np.sqrt(DIM))).astype(np.float32)
    kw_p = np.ascontiguousarray(kw[:, perm])
    vw_p = np.ascontiguousarray(vw[:, perm])
    ow_p = np.ascontiguousarray(ow[perm, :])

    kp = np.arange(128)[:, None]
    qf = np.arange(QCH)[None, :]
    masks = np.stack([(128 * i + kp <= qf) for i in range(4)]).astype(np.float32)

    in_maps = []
    for c in range(N_CORES):
        b, hg = c // 4, c % 4
        csl = slice(hg * 256, (hg + 1) * 256)
        in_maps.append({
            "xT": np.ascontiguousarray(x[b].T),
            "qw": np.ascontiguousarray(qw_p[:, csl]),
            "kw": np.ascontiguousarray(kw_p[:, csl]),
            "vw": np.ascontiguousarray(vw_p[:, csl]),
            "ow": np.ascontiguousarray(ow_p[csl, :]),
            "masks": masks,
        })
    return in_maps


def kernel(x, qw, kw, vw, ow, _trace=False):
    from concourse.bass_utils import run_bass_kernel_spmd

    if _trace:
        _install_ntff_hook()

    nc = _get_nc()
    in_maps = _prep_inputs(x, qw, kw, vw, ow)
    res = run_bass_kernel_spmd(nc, in_maps, core_ids=list(range(N_CORES)),
                               trace=_trace)
    parts = [r["out"] for r in res.results]
    outb = [parts[0] + parts[1] + parts[2] + parts[3],
            parts[4] + parts[5] + parts[6] + parts[7]]
    full = np.stack(outb).astype(np.float32)
    if _trace:
        kernel.last_results = res
        if res.exec_time_ns is not None:
            print(f"HW exec time: {res.exec_time_ns} ns")
        if res.instructions_and_trace:
            print(f"trace: {res.instructions_and_trace[1]}")
    return full


def _install_ntff_hook():
    """The image's antenv lacks axon_hooks; synthesize it so trace=True works."""
    import types
    if 'antenv.axon_hooks' in sys.modules:
        return
    mod = types.ModuleType('antenv.axon_hooks')
    mod._hook = None
    mod.set_axon_ntff_profile_hook = lambda h: setattr(mod, '_hook', h)
    mod.get_axon_ntff_profile_hook = lambda: mod._hook
    sys.modules['antenv.axon_hooks'] = mod
    import antenv
    antenv.axon_hooks = mod
    from trn_agent_boot.trn_boot import _ntff_profile_via_ctypes
    mod.set_axon_ntff_profile_hook(
        _ntff_profile_via_ctypes('/opt/axon/libaxon_pjrt.so'))



# revision 16
# speedup vs baseline: 1.0902x; 1.0902x over previous
"""Multi-head causal attention (B=2, S=2048, D=1024, H=16) on 8 TRN2 cores.

Sharding: core = (batch, group-of-4-heads). Each core computes attention for
its 4 heads of its batch and a rank-256 partial of the output projection;
the host sums the 4 partials per batch. The interleaved head split of the
reference (head h = columns h::16) is undone on the host by permuting the
weight matrices, so on-chip everything is head-contiguous.

v2 (all matmul inputs bf16, output bf16, PE kept hot):
  warmup   ~36 dummy matmuls on the (tiny, first-DMA'd) triangle mask keep
           the HAM clock gate at 2.4 GHz while x/weight DMAs land
  QT/KT    [128, 2048] head-pair-stacked transposed Q/K
  V        projected directly into [k, hd] layout (lhsT = x tiles), no PE
           transposes: Vaug[128, 16, 4, 65] with a ones column for l
  S^T      [128, 2, 512] per (pair, k-tile) PSUM; diagonal k-tiles compute
           only the valid q range (causal trim); triangle mask is one
           [128,2,128] bf16 DVE multiply on the true-diagonal block only
  exp      one ScalarE activation per k-tile over both heads' trimmed scores
  PV       oAB[65, 2, 512] += Vaug^T @ P^T; row 64 accumulates l
  norm     reciprocal_approx_fast on the PSUM l-row, ones[1,128] (x) linv
           broadcast matmul, fused normalize-copy into AT (bf16)
  out      partial = AT.T @ ow accumulated over the 2 pairs in PSUM,
           written to DRAM as bf16 (host upcasts and sums partials)
"""
import sys
sys.path.insert(0, '/opt/trn_rl_repo')

import numpy as np

DIM = 1024
HEADS = 16
S = 2048
B = 2
HD = 64
N_CORES = 8
HPC = 4          # heads per core
PAIRS = 2        # processed as 2 pairs of heads (pair packs the 128-wide PE)
QCH = 512        # q chunk
NKT = S // 128   # k tiles per sequence
NST = S // 128   # s tiles (for V projection)
NWARM = 36       # PE warmup matmuls during input DMA

_nc_cache = None


def _build(debug=False):
    import concourse.bass as bass
    import concourse.tile as tile
    import concourse.mybir as mybir
    from concourse import bacc
    from concourse.dve_ops import (
        RECIP_APPROX_FAST_CONSTS as RECIP_CONSTS,
        RECIPROCAL_APPROX_FAST as RECIP_OP,
    )
    from contextlib import ExitStack

    f32 = mybir.dt.float32
    f32r = mybir.dt.float32r
    bf16 = mybir.dt.bfloat16
    Exp = mybir.ActivationFunctionType.Exp

    nc = bacc.Bacc("TRN2", target_bir_lowering=False, debug=False,
                   enable_asserts=False, num_devices=N_CORES)

    xT = nc.dram_tensor("xT", [DIM, S], bf16, kind="ExternalInput").ap()
    qw = nc.dram_tensor("qw", [DIM, 256], bf16, kind="ExternalInput").ap()
    kw = nc.dram_tensor("kw", [DIM, 256], bf16, kind="ExternalInput").ap()
    vw = nc.dram_tensor("vw", [DIM, 256], bf16, kind="ExternalInput").ap()
    ow = nc.dram_tensor("ow", [256, DIM], bf16, kind="ExternalInput").ap()
    tri = nc.dram_tensor("tri", [128, 128], bf16, kind="ExternalInput").ap()
    out = nc.dram_tensor("out", [S, DIM], bf16, kind="ExternalOutput").ap()
    if debug:
        dbg = {
            "dQT0": nc.dram_tensor("dQT0", [128, S], bf16, kind="ExternalOutput").ap(),
            "dKT0": nc.dram_tensor("dKT0", [128, S], bf16, kind="ExternalOutput").ap(),
            "dVaug": nc.dram_tensor("dVaug", [128, NST, HPC, HD + 1], bf16,
                                    kind="ExternalOutput").ap(),
            "dAT0": nc.dram_tensor("dAT0", [128, S], bf16, kind="ExternalOutput").ap(),
            "dlinv": nc.dram_tensor("dlinv", [8, 2, QCH], bf16,
                                    kind="ExternalOutput").ap(),
            "dlinvB": nc.dram_tensor("dlinvB", [8, 2, QCH], f32,
                                     kind="ExternalOutput").ap(),
            "dlinvC": nc.dram_tensor("dlinvC", [8, 2, QCH], bf16,
                                     kind="ExternalOutput").ap(),
            "dlinvD": nc.dram_tensor("dlinvD", [8, 2, QCH], f32,
                                     kind="ExternalOutput").ap(),
            "dRbr": nc.dram_tensor("dRbr", [128, 2, QCH], f32,
                                   kind="ExternalOutput").ap(),
        }

    with tile.TileContext(nc) as tc, ExitStack() as ctx:
        const_pool = ctx.enter_context(tc.tile_pool(name="const", bufs=1))
        big_pool = ctx.enter_context(tc.tile_pool(name="big", bufs=1))
        pt_pool = ctx.enter_context(tc.tile_pool(name="pt", bufs=4))
        small_pool = ctx.enter_context(tc.tile_pool(name="small", bufs=4))
        outst_pool = ctx.enter_context(tc.tile_pool(name="outst", bufs=3))
        psum_s = ctx.enter_context(tc.tile_pool(name="psum_s", bufs=2, space="PSUM"))
        psum_o = ctx.enter_context(tc.tile_pool(name="psum_o", bufs=2, space="PSUM"))

        # ---- input DMAs, cheapest-first so the PE can warm up, then start
        # ---- projections as soon as (qw, x chunk 0) land
        tri_sb = const_pool.tile([128, 128], bf16, tag="tri")
        nc.sync.dma_start(out=tri_sb, in_=tri)
        qw_sb = const_pool.tile([128, 8, 256], bf16, tag="qw")
        nc.sync.dma_start(out=qw_sb, in_=qw.rearrange("(kt p) m -> p kt m", p=128))
        xin = [const_pool.tile([128, 8, QCH], bf16, tag=f"x{j}", name=f"x{j}")
               for j in range(S // QCH)]
        xTr = xT.rearrange("(kt p) n -> p kt n", p=128)
        nc.sync.dma_start(out=xin[0], in_=xTr[:, :, 0:QCH])
        kw_sb = const_pool.tile([128, 8, 256], bf16, tag="kw")
        nc.sync.dma_start(out=kw_sb, in_=kw.rearrange("(kt p) m -> p kt m", p=128))
        vw_sb = const_pool.tile([128, 8, 256], bf16, tag="vw")
        nc.sync.dma_start(out=vw_sb, in_=vw.rearrange("(kt p) m -> p kt m", p=128))
        nc.sync.dma_start(out=xin[1], in_=xTr[:, :, QCH:2 * QCH])
        ow_sb = const_pool.tile([128, 2, DIM], bf16, tag="ow")
        nc.sync.dma_start(out=ow_sb, in_=ow.rearrange("(t p) n -> p t n", p=128))
        nc.sync.dma_start(out=xin[2], in_=xTr[:, :, 2 * QCH:3 * QCH])
        nc.sync.dma_start(out=xin[3], in_=xTr[:, :, 3 * QCH:4 * QCH])

        ones_r = const_pool.tile([1, 128], bf16, tag="ones_r")
        nc.vector.memset(ones_r, 1.0)

        QT = [big_pool.tile([128, S], bf16, tag=f"QT{p}", name=f"QT{p}")
              for p in range(PAIRS)]
        KT = [big_pool.tile([128, S], bf16, tag=f"KT{p}", name=f"KT{p}")
              for p in range(PAIRS)]
        AT = [big_pool.tile([128, S], bf16, tag=f"AT{p}", name=f"AT{p}")
              for p in range(PAIRS)]
        # Vaug[k, st, h, 0:64] = V_h[k of s-tile st, :]; col 64 = ones (-> l)
        Vaug = big_pool.tile([128, NST, HPC, HD + 1], bf16, tag="Vaug",
                             name="Vaug")
        nc.vector.memset(Vaug[:, :, :, HD:HD + 1], 1.0)

        # ---- PE warmup: keep the HAM clock gate fed while inputs stream in
        wps = psum_s.tile([128, 128], f32, tag="s2", name="warm")
        for _ in range(NWARM):
            nc.tensor.matmul(wps, tri_sb, tri_sb, start=True, stop=True)

        # ---- projection pieces, fed into the attention loop so ScalarE exp
        # ---- work overlaps projection PE work
        def proj_qk(wsb, dstT, j):
            qsl = slice(j * QCH, (j + 1) * QCH)
            ps2 = psum_s.tile([128, 2, QCH], f32, tag="s2", name="ps2")
            for ct in range(PAIRS):
                for kt in range(8):
                    nc.tensor.matmul(
                        ps2[:, ct, :],
                        wsb[:, kt, ct * 128:(ct + 1) * 128],
                        xin[j][:, kt, :],
                        start=(kt == 0), stop=(kt == 7))
            for ct in range(PAIRS):
                nc.vector.tensor_copy(out=dstT[ct][:, qsl], in_=ps2[:, ct, :])

        def proj_v(g):
            # V for s-tiles 2g, 2g+1 directly in [k, hd] layout
            vps = psum_s.tile([128, 2, 256], f32, tag="s2", name="vps")
            for i in range(2):
                st = 2 * g + i
                xj = xin[st // 4]
                ssl = slice((st % 4) * 128, (st % 4) * 128 + 128)
                for kt in range(8):
                    nc.tensor.matmul(
                        vps[:, i, :],
                        xj[:, kt, ssl],
                        vw_sb[:, kt, :],
                        start=(kt == 0), stop=(kt == 7))
            nc.vector.tensor_copy(
                out=Vaug[:, 2 * g:2 * g + 2, :, 0:HD],
                in_=vps.rearrange("p i (h c) -> p i h c", h=HPC))

        def round_feed(j):
            return [lambda: proj_qk(qw_sb, QT, j),
                    lambda: proj_qk(kw_sb, KT, j),
                    lambda: proj_v(2 * j),
                    lambda: proj_v(2 * j + 1)]

        # prologue: round 0 inputs emitted directly
        for piece in round_feed(0):
            piece()

        # ---- Phase 2: causal attention, pair-packed; then output proj ----
        n_fin = [0]

        def finalize_norm(p, j, oAB):
            qsl = slice(j * QCH, (j + 1) * QCH)
            # custom-DVE ops read garbage from PSUM on real HW (CoreSim
            # accepts it): stage the l row through SBUF on ScalarE, then one
            # custom-DVE reciprocal written directly as bf16 (the fp32
            # wrapper assert only protects the bitwise seed on the input)
            lrows = small_pool.tile([1, 2, QCH], f32, tag="lrows", name="lrows")
            nc.scalar.copy(out=lrows, in_=oAB[HD:HD + 1, :, :])
            linv = small_pool.tile([1, 2, QCH], bf16, tag="linv", name="linv")
            c = RECIP_CONSTS
            nc.vector._custom_dve(RECIP_OP, out=linv, in0=lrows,
                                  s0=c["s0"], s1=c["s1"], imm2=c["imm2"])
            Rbr = psum_s.tile([128, 2, QCH], f32, tag="s2", name="Rbr")
            for hh in range(2):
                nc.tensor.matmul(Rbr[:, hh, :], ones_r, linv[0:1, hh, :],
                                 start=True, stop=True)
            # DVE may read only ONE PSUM operand per op: ACT copies the
            # attention rows out, DVE multiplies in-place by the PSUM
            # broadcast of 1/l
            for hh in range(2):
                nc.scalar.copy(out=AT[p][hh * HD:(hh + 1) * HD, qsl],
                               in_=oAB[0:HD, hh, :])
            for hh in range(2):
                nc.vector.tensor_mul(
                    AT[p][hh * HD:(hh + 1) * HD, qsl],
                    AT[p][hh * HD:(hh + 1) * HD, qsl],
                    Rbr[hh * HD:(hh + 1) * HD, hh, :])
            if debug:
                nc.sync.dma_start(out=dbg["dlinv"][n_fin[0]:n_fin[0] + 1],
                                  in_=linv)
                n_fin[0] += 1

        def emit_outproj(j):
            # output projection for q rows of chunk j (needs both pairs' AT)
            for rt in range(QCH // 128):
                rsl = slice(j * QCH + rt * 128, j * QCH + (rt + 1) * 128)
                po2 = psum_s.tile([128, 2, QCH], f32, tag="s2", name="po2")
                for nch in range(DIM // QCH):
                    for pp in range(PAIRS):
                        nc.tensor.matmul(
                            po2[:, nch, :], AT[pp][:, rsl],
                            ow_sb[:, pp, nch * QCH:(nch + 1) * QCH],
                            start=(pp == 0), stop=(pp == PAIRS - 1))
                ot = outst_pool.tile([128, 2, QCH], bf16, tag="ot", name="ot")
                if rt % 2 == 0:
                    nc.vector.tensor_copy(out=ot, in_=po2)
                else:
                    nc.scalar.copy(out=ot, in_=po2)
                nc.sync.dma_start(
                    out=out[rsl, :].rearrange("p (a b) -> p a b", a=2),
                    in_=ot)

        def tri_bc(n):
            # tri_sb broadcast over the hh axis (stride-0 free dim)
            return bass.AP(tensor=tri_sb.tensor, offset=tri_sb.offset,
                           ap=[list(tri_sb.ap[0]), [0, n]] +
                              [list(d) for d in tri_sb.ap[1:]])

        def attend_kt(p, j, kt, oAB, pv_prev):
            # S^T + exp (+ triangle mask) for (pair, ktile); PV of prev ktile
            di = kt - 4 * j
            q0 = 128 * di if di > 0 else 0
            ksl = slice(kt * 128, (kt + 1) * 128)
            sAB = psum_s.tile([128, 2, QCH], f32, tag="s2", name="sAB")
            for hh in range(2):
                nc.tensor.matmul(sAB[:, hh, q0:QCH],
                                 KT[p][hh * 64:(hh + 1) * 64, ksl],
                                 QT[p][hh * 64:(hh + 1) * 64,
                                       j * QCH + q0:(j + 1) * QCH],
                                 start=True, stop=True)
            pAB = pt_pool.tile([128, 2, QCH], bf16, tag="pt", name="pAB")
            nc.scalar.activation(out=pAB[:, :, q0:QCH], in_=sAB[:, :, q0:QCH],
                                 func=Exp)
            if di >= 0:
                nc.vector.tensor_mul(pAB[:, :, q0:q0 + 128],
                                     pAB[:, :, q0:q0 + 128], tri_bc(2))
            if pv_prev is not None:
                emit_pv(p, j, oAB, pv_prev, stop=False)
            return (kt, q0, pAB)

        def emit_pv(p, j, oAB, pv, stop):
            kprev, q0p, pprev = pv
            for hh in range(2):
                nc.tensor.matmul(oAB[:, hh, q0p:QCH],
                                 Vaug[:, kprev, 2 * p + hh, :],
                                 pprev[:, hh, q0p:QCH],
                                 start=(kprev == 0), stop=stop)

        pending_norm = None
        pending_proj = None
        feed = []
        NJ = S // QCH
        for j in range(NJ - 1):
            feed = round_feed(j + 1)
            for p in range(PAIRS):
                nkt = 4 * (j + 1)
                if pending_norm is not None and j != 0:
                    finalize_norm(*pending_norm)
                    pending_norm = None
                all_diag = (j == 0)
                oAB = psum_o.tile([HD + 1, 2, QCH], f32, tag="o", name="oAB")
                pv_prev = None
                for kt in range(nkt):
                    pv_prev = attend_kt(p, j, kt, oAB, pv_prev)
                    if feed:
                        feed.pop(0)()
                    if (pending_norm is not None and not all_diag
                            and kt == min(3, nkt - 1)):
                        finalize_norm(*pending_norm)
                        pending_norm = None
                    if pending_proj is not None and kt == min(9, nkt - 1):
                        emit_outproj(pending_proj)
                        pending_proj = None
                emit_pv(p, j, oAB, pv_prev, stop=True)
                if pending_norm is not None:
                    finalize_norm(*pending_norm)
                pending_norm = (p, j, oAB)
                if p == PAIRS - 1:
                    pending_proj = j
            while feed:
                feed.pop(0)()

        # last chunk: interleave the two pairs per ktile so both finish
        # together and the tail (finalize + outproj + out DMA) is short.
        # The pending pair must be finalized BEFORE the two oAB allocations
        # below: both psum_o buffers are needed, and deferring the finalize
        # would put its slot-freeing work behind the PE instructions that
        # wait on the slot (cross-engine deadlock).
        j = NJ - 1
        nkt = 4 * NJ
        if pending_norm is not None:
            finalize_norm(*pending_norm)
            pending_norm = None
        oABs = [psum_o.tile([HD + 1, 2, QCH], f32, tag="o", name=f"oL{p}")
                for p in range(PAIRS)]
        pvp = [None, None]
        for kt in range(nkt):
            for p in range(PAIRS):
                pvp[p] = attend_kt(p, j, kt, oABs[p], pvp[p])
            if pending_proj is not None and kt == 9:
                emit_outproj(pending_proj)
                pending_proj = None
        for p in range(PAIRS):
            emit_pv(p, j, oABs[p], pvp[p], stop=True)
            finalize_norm(p, j, oABs[p])
        emit_outproj(j)
        if debug:
            nc.sync.dma_start(out=dbg["dQT0"], in_=QT[0])
            nc.sync.dma_start(out=dbg["dKT0"], in_=KT[0])
            nc.sync.dma_start(out=dbg["dVaug"], in_=Vaug)
            nc.sync.dma_start(out=dbg["dAT0"], in_=AT[0])

    nc.compile()
    return nc


def _get_nc():
    global _nc_cache
    if _nc_cache is None:
        _nc_cache = _build()
    return _nc_cache


def _prep_inputs(x, qw, kw, vw, ow):
    import ml_dtypes
    bf16 = ml_dtypes.bfloat16
    # undo interleaved head split: head h = cols h::16 -> contiguous blocks
    perm = np.concatenate([np.arange(h, DIM, HEADS) for h in range(HEADS)])
    qw_p = (qw[:, perm] / np.float32(np.sqrt(DIM))).astype(bf16)
    kw_p = kw[:, perm].astype(bf16)
    vw_p = vw[:, perm].astype(bf16)
    ow_p = ow[perm, :].astype(bf16)

    kp = np.arange(128)[:, None]
    qf = np.arange(128)[None, :]
    tri = (kp <= qf).astype(bf16)

    in_maps = []
    for c in range(N_CORES):
        b, hg = c // 4, c % 4
        csl = slice(hg * 256, (hg + 1) * 256)
        in_maps.append({
            "xT": np.ascontiguousarray(x[b].T).astype(bf16),
            "qw": np.ascontiguousarray(qw_p[:, csl]),
            "kw": np.ascontiguousarray(kw_p[:, csl]),
            "vw": np.ascontiguousarray(vw_p[:, csl]),
            "ow": np.ascontiguousarray(ow_p[csl, :]),
            "tri": tri,
        })
    return in_maps


def kernel(x, qw, kw, vw, ow, _trace=False):
    from concourse.bass_utils import run_bass_kernel_spmd

    if _trace:
        _install_ntff_hook()

    nc = _get_nc()
    in_maps = _prep_inputs(x, qw, kw, vw, ow)
    res = run_bass_kernel_spmd(nc, in_maps, core_ids=list(range(N_CORES)),
                               trace=_trace)
    parts = [r["out"].astype(np.float32) for r in res.results]
    outb = [parts[0] + parts[1] + parts[2] + parts[3],
            parts[4] + parts[5] + parts[6] + parts[7]]
    full = np.stack(outb).astype(np.float32)
    if _trace:
        kernel.last_results = res
        if res.exec_time_ns is not None:
            print(f"HW exec time: {res.exec_time_ns} ns")
        if res.instructions_and_trace:
            print(f"trace: {res.instructions_and_trace[1]}")
    return full


def _install_ntff_hook():
    """The image's antenv lacks axon_hooks; synthesize it so trace=True works."""
    import types
    if 'antenv.axon_hooks' in sys.modules:
        return
    mod = types.ModuleType('antenv.axon_hooks')
    mod._hook = None
    mod.set_axon_ntff_profile_hook = lambda h: setattr(mod, '_hook', h)
    mod.get_axon_ntff_profile_hook = lambda: mod._hook
    sys.modules['antenv.axon_hooks'] = mod
    import antenv
    antenv.axon_hooks = mod
    from trn_agent_boot.trn_boot import _ntff_profile_via_ctypes
    mod.set_axon_ntff_profile_hook(
        _ntff_profile_via_ctypes('/opt/axon/libaxon_pjrt.so'))


# revision 31
# speedup vs baseline: 1.2592x; 1.1549x over previous
"""Multi-head causal attention (B=2, S=2048, D=1024, H=16) on 8 TRN2 cores.

Sharding: core = (batch, group-of-4-heads). Each core computes attention for
its 4 heads of its batch and a rank-256 partial of the output projection;
the host sums the 4 partials per batch. The interleaved head split of the
reference (head h = columns h::16) is undone on the host by permuting the
weight matrices, so on-chip everything is head-contiguous.

v2 (all matmul inputs bf16, output bf16, PE kept hot):
  warmup   ~36 dummy matmuls on the (tiny, first-DMA'd) triangle mask keep
           the HAM clock gate at 2.4 GHz while x/weight DMAs land
  QT/KT    [128, 2048] head-pair-stacked transposed Q/K
  V        projected directly into [k, hd] layout (lhsT = x tiles), no PE
           transposes: Vaug[128, 16, 4, 65] with a ones column for l
  S^T      [128, 2, 512] per (pair, k-tile) PSUM; diagonal k-tiles compute
           only the valid q range (causal trim); triangle mask is one
           [128,2,128] bf16 DVE multiply on the true-diagonal block only
  exp      one ScalarE activation per k-tile over both heads' trimmed scores
  PV       oAB[65, 2, 512] += Vaug^T @ P^T; row 64 accumulates l
  norm     reciprocal_approx_fast on the PSUM l-row, ones[1,128] (x) linv
           broadcast matmul, fused normalize-copy into AT (bf16)
  out      partial = AT.T @ ow accumulated over the 2 pairs in PSUM,
           written to DRAM as bf16 (host upcasts and sums partials)
"""
import sys
sys.path.insert(0, '/opt/trn_rl_repo')

import numpy as np

DIM = 1024
HEADS = 16
S = 2048
B = 2
HD = 64
N_CORES = 8
HPC = 4          # heads per core
PAIRS = 2        # processed as 2 pairs of heads (pair packs the 128-wide PE)
QCH = 512        # q chunk
NKT = S // 128   # k tiles per sequence
NST = S // 128   # s tiles (for V projection)
NWARM = 44       # PE warmup matmuls during input DMA

_nc_cache = None


def _build(debug=False):
    import concourse.bass as bass
    import concourse.tile as tile
    import concourse.mybir as mybir
    from concourse import bacc
    from concourse.dve_ops import (
        RECIP_APPROX_FAST_CONSTS as RECIP_CONSTS,
        RECIPROCAL_APPROX_FAST as RECIP_OP,
    )
    from concourse.masks import make_upper_triangular
    from contextlib import ExitStack

    f32 = mybir.dt.float32
    f32r = mybir.dt.float32r
    bf16 = mybir.dt.bfloat16
    Exp = mybir.ActivationFunctionType.Exp

    nc = bacc.Bacc("TRN2", target_bir_lowering=False, debug=False,
                   enable_asserts=False, num_devices=N_CORES)

    xT = nc.dram_tensor("xT", [DIM, S], bf16, kind="ExternalInput").ap()
    qw = nc.dram_tensor("qw", [DIM, 256], bf16, kind="ExternalInput").ap()
    kw = nc.dram_tensor("kw", [DIM, 256], bf16, kind="ExternalInput").ap()
    vw = nc.dram_tensor("vw", [DIM, 256], bf16, kind="ExternalInput").ap()
    ow = nc.dram_tensor("ow", [256, DIM], bf16, kind="ExternalInput").ap()
    out = nc.dram_tensor("out", [S, DIM], bf16, kind="ExternalOutput").ap()
    if debug:
        dbg = {
            "dQT0": nc.dram_tensor("dQT0", [128, S], bf16, kind="ExternalOutput").ap(),
            "dKT0": nc.dram_tensor("dKT0", [128, S], bf16, kind="ExternalOutput").ap(),
            "dVaug": nc.dram_tensor("dVaug", [128, NST, HPC, HD + 1], bf16,
                                    kind="ExternalOutput").ap(),
            "dAT0": nc.dram_tensor("dAT0", [128, S], bf16, kind="ExternalOutput").ap(),
            "dlinv": nc.dram_tensor("dlinv", [8, 2, QCH], bf16,
                                    kind="ExternalOutput").ap(),
            "dlinvB": nc.dram_tensor("dlinvB", [8, 2, QCH], f32,
                                     kind="ExternalOutput").ap(),
            "dlinvC": nc.dram_tensor("dlinvC", [8, 2, QCH], bf16,
                                     kind="ExternalOutput").ap(),
            "dlinvD": nc.dram_tensor("dlinvD", [8, 2, QCH], f32,
                                     kind="ExternalOutput").ap(),
            "dRbr": nc.dram_tensor("dRbr", [128, 2, QCH], f32,
                                   kind="ExternalOutput").ap(),
        }

    with tile.TileContext(nc) as tc, ExitStack() as ctx:
        const_pool = ctx.enter_context(tc.tile_pool(name="const", bufs=1))
        big_pool = ctx.enter_context(tc.tile_pool(name="big", bufs=1))
        pt_pool = ctx.enter_context(tc.tile_pool(name="pt", bufs=6))
        small_pool = ctx.enter_context(tc.tile_pool(name="small", bufs=4))
        outst_pool = ctx.enter_context(tc.tile_pool(name="outst", bufs=3))
        psum_s = ctx.enter_context(tc.tile_pool(name="psum_s", bufs=2, space="PSUM"))
        psum_o = ctx.enter_context(tc.tile_pool(name="psum_o", bufs=2, space="PSUM"))

        # ---- triangle mask generated on-device (GpSimd): tri[k, q] = k <= q.
        # No DMA needed, so PE warmup matmuls on it can start immediately.
        tri_sb = const_pool.tile([128, 128], bf16, tag="tri")
        make_upper_triangular(nc, tri_sb, val=1.0, diag=True)

        # ---- input DMAs ordered so projections start as soon as possible
        qw_sb = const_pool.tile([128, 8, 256], bf16, tag="qw")
        nc.sync.dma_start(out=qw_sb, in_=qw.rearrange("(kt p) m -> p kt m", p=128))
        xin = [const_pool.tile([128, 8, QCH], bf16, tag=f"x{j}", name=f"x{j}")
               for j in range(S // QCH)]
        xTr = xT.rearrange("(kt p) n -> p kt n", p=128)
        nc.sync.dma_start(out=xin[0], in_=xTr[:, :, 0:QCH])
        kw_sb = const_pool.tile([128, 8, 256], bf16, tag="kw")
        nc.sync.dma_start(out=kw_sb, in_=kw.rearrange("(kt p) m -> p kt m", p=128))
        vw_sb = const_pool.tile([128, 8, 256], bf16, tag="vw")
        nc.sync.dma_start(out=vw_sb, in_=vw.rearrange("(kt p) m -> p kt m", p=128))
        nc.sync.dma_start(out=xin[1], in_=xTr[:, :, QCH:2 * QCH])
        ow_sb = const_pool.tile([128, 2, DIM], bf16, tag="ow")
        nc.sync.dma_start(out=ow_sb, in_=ow.rearrange("(t p) n -> p t n", p=128))
        nc.sync.dma_start(out=xin[2], in_=xTr[:, :, 2 * QCH:3 * QCH])
        nc.sync.dma_start(out=xin[3], in_=xTr[:, :, 3 * QCH:4 * QCH])

        ones_r = const_pool.tile([1, 128], bf16, tag="ones_r")
        nc.vector.memset(ones_r, 1.0)

        QTa = big_pool.tile([128, PAIRS, S], bf16, tag="QTa", name="QTa")
        KTa = big_pool.tile([128, PAIRS, S], bf16, tag="KTa", name="KTa")
        AT = [big_pool.tile([128, S], bf16, tag=f"AT{p}", name=f"AT{p}")
              for p in range(PAIRS)]
        # Vaug[k, st, h, 0:64] = V_h[k of s-tile st, :]; col 64 = ones (-> l)
        Vaug = big_pool.tile([128, NST, HPC, HD + 1], bf16, tag="Vaug",
                             name="Vaug")
        nc.vector.memset(Vaug[:, :, :, HD:HD + 1], 1.0)

        # ---- PE warmup: keep the HAM clock gate fed while inputs stream in
        wps = psum_s.tile([128, 128], f32, tag="s2", name="warm")
        for _ in range(NWARM):
            nc.tensor.matmul(wps, tri_sb, tri_sb, start=True, stop=True)

        # ---- projection pieces, fed into the attention loop so ScalarE exp
        # ---- work overlaps projection PE work
        def proj_qk(wsb, dstT, j):
            qsl = slice(j * QCH, (j + 1) * QCH)
            ps2 = psum_s.tile([128, 2, QCH], f32, tag="s2", name="ps2")
            for ct in range(PAIRS):
                for kt in range(8):
                    nc.tensor.matmul(
                        ps2[:, ct, :],
                        wsb[:, kt, ct * 128:(ct + 1) * 128],
                        xin[j][:, kt, :],
                        start=(kt == 0), stop=(kt == 7))
            nc.vector.tensor_copy(out=dstT[:, :, qsl], in_=ps2)

        def proj_v(g):
            # V for s-tiles 2g, 2g+1 directly in [k, hd] layout
            vps = psum_s.tile([128, 2, 256], f32, tag="s2", name="vps")
            for i in range(2):
                st = 2 * g + i
                xj = xin[st // 4]
                ssl = slice((st % 4) * 128, (st % 4) * 128 + 128)
                for kt in range(8):
                    nc.tensor.matmul(
                        vps[:, i, :],
                        xj[:, kt, ssl],
                        vw_sb[:, kt, :],
                        start=(kt == 0), stop=(kt == 7))
            nc.vector.tensor_copy(
                out=Vaug[:, 2 * g:2 * g + 2, :, 0:HD],
                in_=vps.rearrange("p i (h c) -> p i h c", h=HPC))

        def round_feed(j):
            return [lambda: proj_qk(qw_sb, QTa, j),
                    lambda: proj_qk(kw_sb, KTa, j),
                    lambda: proj_v(2 * j),
                    lambda: proj_v(2 * j + 1)]

        # prologue: round 0 inputs emitted directly
        for piece in round_feed(0):
            piece()

        # ---- Phase 2: causal attention, pair-packed; then output proj ----
        n_fin = [0]

        def finalize_norm(p, j, oAB):
            qsl = slice(j * QCH, (j + 1) * QCH)
            # custom-DVE ops read garbage from PSUM on real HW (CoreSim
            # accepts it): stage the l row through SBUF, then one custom-DVE
            # reciprocal written directly as bf16 (the fp32 wrapper assert
            # only protects the bitwise seed on the input)
            lrows = small_pool.tile([1, 2, QCH], f32, tag="lrows", name="lrows")
            nc.vector.tensor_copy(out=lrows, in_=oAB[HD:HD + 1, :, :])
            linv = small_pool.tile([1, 2, QCH], bf16, tag="linv", name="linv")
            c = RECIP_CONSTS
            nc.vector._custom_dve(RECIP_OP, out=linv, in0=lrows,
                                  s0=c["s0"], s1=c["s1"], imm2=c["imm2"])
            Rbr = psum_s.tile([128, 2, QCH], f32, tag="s2", name="Rbr")
            for hh in range(2):
                nc.tensor.matmul(Rbr[:, hh, :], ones_r, linv[0:1, hh, :],
                                 start=True, stop=True)
            # DVE may read only ONE PSUM operand per op: stage the 1/l
            # broadcast to SBUF once, then normalize straight out of oAB
            Rsb = small_pool.tile([128, 2, QCH], f32, tag="Rsb", name="Rsb")
            nc.vector.tensor_copy(out=Rsb, in_=Rbr)
            for hh in range(2):
                nc.vector.tensor_mul(
                    AT[p][hh * HD:(hh + 1) * HD, qsl],
                    oAB[0:HD, hh, :],
                    Rsb[hh * HD:(hh + 1) * HD, hh, :])
            if debug:
                nc.sync.dma_start(out=dbg["dlinv"][n_fin[0]:n_fin[0] + 1],
                                  in_=linv)
                n_fin[0] += 1

        def outproj_rt(j, rt):
            # output projection for one 128-row tile of chunk j
            rsl = slice(j * QCH + rt * 128, j * QCH + (rt + 1) * 128)
            po2 = psum_s.tile([128, 2, QCH], f32, tag="s2", name="po2")
            for nch in range(DIM // QCH):
                for pp in range(PAIRS):
                    nc.tensor.matmul(
                        po2[:, nch, :], AT[pp][:, rsl],
                        ow_sb[:, pp, nch * QCH:(nch + 1) * QCH],
                        start=(pp == 0), stop=(pp == PAIRS - 1))
            ot = outst_pool.tile([128, 2, QCH], bf16, tag="ot", name="ot")
            nc.vector.tensor_copy(out=ot, in_=po2)
            nc.sync.dma_start(
                out=out[rsl, :].rearrange("p (a b) -> p a b", a=2),
                in_=ot)

        def outproj_pieces(j):
            return [lambda rt=rt: outproj_rt(j, rt) for rt in range(QCH // 128)]

        def tri_bc(n):
            # tri_sb broadcast over the hh axis (stride-0 free dim)
            return bass.AP(tensor=tri_sb.tensor, offset=tri_sb.offset,
                           ap=[list(tri_sb.ap[0]), [0, n]] +
                              [list(d) for d in tri_sb.ap[1:]])

        def attend_kt(p, j, kt, oAB, pv_prev):
            # S^T + exp (+ triangle mask) for (pair, ktile); PV of prev ktile
            di = kt - 4 * j
            q0 = 128 * di if di > 0 else 0
            ksl = slice(kt * 128, (kt + 1) * 128)
            sAB = psum_s.tile([128, 2, QCH], f32, tag="s2", name="sAB")
            for hh in range(2):
                nc.tensor.matmul(sAB[:, hh, q0:QCH],
                                 KTa[hh * 64:(hh + 1) * 64, p, ksl],
                                 QTa[hh * 64:(hh + 1) * 64, p,
                                     j * QCH + q0:(j + 1) * QCH],
                                 start=True, stop=True)
            pAB = pt_pool.tile([128, 2, QCH], bf16, tag="pt", name="pAB")
            nc.scalar.activation(out=pAB[:, :, q0:QCH], in_=sAB[:, :, q0:QCH],
                                 func=Exp)
            if di >= 0:
                # triangle mask on the (otherwise idle) GpSimd engine
                nc.gpsimd.tensor_mul(pAB[:, :, q0:q0 + 128],
                                     pAB[:, :, q0:q0 + 128], tri_bc(2))
            if pv_prev is not None:
                emit_pv(p, j, oAB, pv_prev, stop=False)
            return (kt, q0, pAB)

        def emit_pv(p, j, oAB, pv, stop):
            kprev, q0p, pprev = pv
            for hh in range(2):
                nc.tensor.matmul(oAB[:, hh, q0p:QCH],
                                 Vaug[:, kprev, 2 * p + hh, :],
                                 pprev[:, hh, q0p:QCH],
                                 start=(kprev == 0), stop=stop)

        pending_norm = None
        NJ = S // QCH
        feedq, ojq = [], []
        for j in range(NJ - 1):
            feedq = round_feed(j + 1)
            for p in range(PAIRS):
                nkt = 4 * (j + 1)
                if pending_norm is not None and j != 0:
                    finalize_norm(*pending_norm)
                    pending_norm = None
                all_diag = (j == 0)
                oAB = psum_o.tile([HD + 1, 2, QCH], f32, tag="o", name="oAB")
                pv_prev = None
                for kt in range(nkt):
                    pv_prev = attend_kt(p, j, kt, oAB, pv_prev)
                    if feedq:
                        feedq.pop(0)()
                    elif ojq and (kt >= 2 or p == 1):
                        ojq.pop(0)()
                    if (pending_norm is not None and not all_diag
                            and kt == min(3, nkt - 1)):
                        finalize_norm(*pending_norm)
                        pending_norm = None
                emit_pv(p, j, oAB, pv_prev, stop=True)
                if pending_norm is not None:
                    finalize_norm(*pending_norm)
                pending_norm = (p, j, oAB)
            while feedq:
                feedq.pop(0)()
            while ojq:
                ojq.pop(0)()
            ojq = outproj_pieces(j)

        # last chunk: interleave the two pairs per ktile so both finish
        # together and the tail (finalize + outproj + out DMA) is short.
        # The pending pair must be finalized BEFORE the two oAB allocations
        # below: both psum_o buffers are needed, and deferring the finalize
        # would put its slot-freeing work behind the PE instructions that
        # wait on the slot (cross-engine deadlock).
        j = NJ - 1
        nkt = 4 * NJ
        if pending_norm is not None:
            finalize_norm(*pending_norm)
            pending_norm = None
        oABs = [psum_o.tile([HD + 1, 2, QCH], f32, tag="o", name=f"oL{p}")
                for p in range(PAIRS)]
        pvp = [None, None]
        for kt in range(nkt):
            for p in range(PAIRS):
                pvp[p] = attend_kt(p, j, kt, oABs[p], pvp[p])
            if ojq and kt >= 1:
                ojq.pop(0)()
        while ojq:
            ojq.pop(0)()
        for p in range(PAIRS):
            emit_pv(p, j, oABs[p], pvp[p], stop=True)
            finalize_norm(p, j, oABs[p])
        for piece in outproj_pieces(j):
            piece()
        if debug:
            nc.sync.dma_start(out=dbg["dQT0"], in_=QTa[:, 0, :])
            nc.sync.dma_start(out=dbg["dKT0"], in_=KTa[:, 0, :])
            nc.sync.dma_start(out=dbg["dVaug"], in_=Vaug)
            nc.sync.dma_start(out=dbg["dAT0"], in_=AT[0])

    nc.compile()
    return nc


def _get_nc():
    global _nc_cache
    if _nc_cache is None:
        _nc_cache = _build()
    return _nc_cache


def _prep_inputs(x, qw, kw, vw, ow):
    import ml_dtypes
    bf16 = ml_dtypes.bfloat16
    # undo interleaved head split: head h = cols h::16 -> contiguous blocks
    perm = np.concatenate([np.arange(h, DIM, HEADS) for h in range(HEADS)])
    qw_p = (qw[:, perm] / np.float32(np.sqrt(DIM))).astype(bf16)
    kw_p = kw[:, perm].astype(bf16)
    vw_p = vw[:, perm].astype(bf16)
    ow_p = ow[perm, :].astype(bf16)

    in_maps = []
    for c in range(N_CORES):
        b, hg = c // 4, c % 4
        csl = slice(hg * 256, (hg + 1) * 256)
        in_maps.append({
            "xT": np.ascontiguousarray(x[b].T).astype(bf16),
            "qw": np.ascontiguousarray(qw_p[:, csl]),
            "kw": np.ascontiguousarray(kw_p[:, csl]),
            "vw": np.ascontiguousarray(vw_p[:, csl]),
            "ow": np.ascontiguousarray(ow_p[csl, :]),
        })
    return in_maps


def kernel(x, qw, kw, vw, ow, _trace=False):
    from concourse.bass_utils import run_bass_kernel_spmd

    if _trace:
        _install_ntff_hook()

    nc = _get_nc()
    in_maps = _prep_inputs(x, qw, kw, vw, ow)
    res = run_bass_kernel_spmd(nc, in_maps, core_ids=list(range(N_CORES)),
                               trace=_trace)
    parts = [r["out"].astype(np.float32) for r in res.results]
    outb = [parts[0] + parts[1] + parts[2] + parts[3],
            parts[4] + parts[5] + parts[6] + parts[7]]
    full = np.stack(outb).astype(np.float32)
    if _trace:
        kernel.last_results = res
        if res.exec_time_ns is not None:
            print(f"HW exec time: {res.exec_time_ns} ns")
        if res.instructions_and_trace:
            print(f"trace: {res.instructions_and_trace[1]}")
    return full


def _install_ntff_hook():
    """The image's antenv lacks axon_hooks; synthesize it so trace=True works."""
    import types
    if 'antenv.axon_hooks' in sys.modules:
        return
    mod = types.ModuleType('antenv.axon_hooks')
    mod._hook = None
    mod.set_axon_ntff_profile_hook = lambda h: setattr(mod, '_hook', h)
    mod.get_axon_ntff_profile_hook = lambda: mod._hook
    sys.modules['antenv.axon_hooks'] = mod
    import antenv
    antenv.axon_hooks = mod
    from trn_agent_boot.trn_boot import _ntff_profile_via_ctypes
    mod.set_axon_ntff_profile_hook(
        _ntff_profile_via_ctypes('/opt/axon/libaxon_pjrt.so'))


# revision 32
# speedup vs baseline: 1.3634x; 1.0828x over previous
"""Multi-head causal attention (B=2, S=2048, D=1024, H=16) on 8 TRN2 cores.

Sharding: core = (batch, group-of-4-heads). Each core computes attention for
its 4 heads of its batch and a rank-256 partial of the output projection;
the host sums the 4 partials per batch. The interleaved head split of the
reference (head h = columns h::16) is undone on the host by permuting the
weight matrices, so on-chip everything is head-contiguous.

v3 (all matmul inputs bf16, output bf16, PE kept hot):
  warmup   dummy matmuls on a memset scratch keep the HAM clock gate at
           2.4 GHz from engine start (~6us) until the first x chunk lands
  DMA      host pre-packs every input so each partition is one contiguous
           segment (1 descriptor per partition); x chunks are chained with
           explicit deps so chunk 0 gets full DMA bandwidth first
  QT/KT    [128, 2, 2048] head-pair-stacked transposed Q/K
  V        projected directly into [k, hd] layout (lhsT = x tiles), no PE
           transposes: Vaug[128, 16, 4, 65] with a ones column for l
  S^T      [128, 2, 512] per (pair, k-tile) PSUM; diagonal k-tiles compute
           only the valid q range (causal trim); triangle mask is one
           [128,2,128] bf16 multiply on the (otherwise idle) GpSimd engine
  exp      one ScalarE activation per k-tile over both heads' trimmed scores
  PV       oAB[65, 2, 512] += Vaug^T @ P^T; row 64 accumulates l
  norm     split in two so the broadcast matmul never head-of-line blocks
           the PE queue: (a) stage l to SBUF + reciprocal_approx_fast right
           after the pair's last PV; (b) ones (x) linv broadcast matmul +
           PSUM->SBUF stage + normalize multiplies, emitted a few k-tiles
           into the next pair
  out      partial = AT.T @ ow accumulated over the 2 pairs in PSUM, one
           128-row tile at a time interleaved into the attention stream,
           written to DRAM as bf16 (host upcasts and sums partials)
"""
import sys
sys.path.insert(0, '/opt/trn_rl_repo')

import numpy as np

DIM = 1024
HEADS = 16
S = 2048
B = 2
HD = 64
N_CORES = 8
HPC = 4          # heads per core
PAIRS = 2        # processed as 2 pairs of heads (pair packs the 128-wide PE)
QCH = 512        # q chunk
NKT = S // 128   # k tiles per sequence
NST = S // 128   # s tiles (for V projection)
NWARM = 64       # PE warmup matmuls during input DMA

_nc_cache = None


def _build(debug=False):
    import concourse.bass as bass
    import concourse.tile as tile
    import concourse.mybir as mybir
    from concourse import bacc
    from concourse.dve_ops import (
        RECIP_APPROX_FAST_CONSTS as RECIP_CONSTS,
        RECIPROCAL_APPROX_FAST as RECIP_OP,
    )
    from concourse.masks import make_upper_triangular
    from concourse.tile_rust import add_dep_helper
    from contextlib import ExitStack

    f32 = mybir.dt.float32
    bf16 = mybir.dt.bfloat16
    Exp = mybir.ActivationFunctionType.Exp

    nc = bacc.Bacc("TRN2", target_bir_lowering=False, debug=False,
                   enable_asserts=False, num_devices=N_CORES)

    # host pre-packed: each [128, ...] with one contiguous run per partition
    xp = nc.dram_tensor("xp", [S // QCH, 128, 8, QCH], bf16,
                        kind="ExternalInput").ap()
    qkv = nc.dram_tensor("qkv", [128, 8, 768], bf16, kind="ExternalInput").ap()
    owp = nc.dram_tensor("owp", [128, 2, DIM], bf16, kind="ExternalInput").ap()
    out = nc.dram_tensor("out", [S, DIM], bf16, kind="ExternalOutput").ap()
    if debug:
        dbg = {
            "dQT0": nc.dram_tensor("dQT0", [128, S], bf16, kind="ExternalOutput").ap(),
            "dKT0": nc.dram_tensor("dKT0", [128, S], bf16, kind="ExternalOutput").ap(),
            "dVaug": nc.dram_tensor("dVaug", [128, NST, HPC, HD + 1], bf16,
                                    kind="ExternalOutput").ap(),
            "dAT0": nc.dram_tensor("dAT0", [128, S], bf16, kind="ExternalOutput").ap(),
        }

    with tile.TileContext(nc) as tc, ExitStack() as ctx:
        const_pool = ctx.enter_context(tc.tile_pool(name="const", bufs=1))
        big_pool = ctx.enter_context(tc.tile_pool(name="big", bufs=1))
        pt_pool = ctx.enter_context(tc.tile_pool(name="pt", bufs=6))
        small_pool = ctx.enter_context(tc.tile_pool(name="small", bufs=4))
        outst_pool = ctx.enter_context(tc.tile_pool(name="outst", bufs=3))
        psum_s = ctx.enter_context(tc.tile_pool(name="psum_s", bufs=2, space="PSUM"))
        psum_o = ctx.enter_context(tc.tile_pool(name="psum_o", bufs=2, space="PSUM"))

        # ---- input DMAs; x chunks chained so chunk 0 gets full bandwidth
        qkv_sb = const_pool.tile([128, 8, 768], bf16, tag="qkv")
        nc.sync.dma_start(out=qkv_sb, in_=qkv)
        xin = [const_pool.tile([128, 8, QCH], bf16, tag=f"x{j}", name=f"x{j}")
               for j in range(S // QCH)]
        dprev = nc.sync.dma_start(out=xin[0], in_=xp[0])
        for j in range(1, S // QCH):
            d = nc.sync.dma_start(out=xin[j], in_=xp[j])
            add_dep_helper(d.ins, dprev.ins,
                           reason="x chunk DMAs chained for chunk-0 latency")
            dprev = d
        ow_sb = const_pool.tile([128, 2, DIM], bf16, tag="ow")
        d = nc.sync.dma_start(out=ow_sb, in_=owp)
        add_dep_helper(d.ins, dprev.ins, reason="ow after x chunks")

        # ---- PE warmup on a memset scratch (no DMA/GpSimd dependency):
        # keeps the HAM clock gate fed until real work arrives
        wsc = const_pool.tile([128, 128], bf16, tag="wsc")
        nc.vector.memset(wsc, 0.5)
        wps = psum_s.tile([128, 128], f32, tag="s2", name="warm")
        for _ in range(NWARM):
            nc.tensor.matmul(wps, wsc, wsc, start=True, stop=True)

        # triangle mask generated on-device (GpSimd): tri[k, q] = k <= q
        tri_sb = const_pool.tile([128, 128], bf16, tag="tri")
        make_upper_triangular(nc, tri_sb, val=1.0, diag=True)

        ones_r = const_pool.tile([1, 128], bf16, tag="ones_r")
        nc.vector.memset(ones_r, 1.0)

        QTa = big_pool.tile([128, PAIRS, S], bf16, tag="QTa", name="QTa")
        KTa = big_pool.tile([128, PAIRS, S], bf16, tag="KTa", name="KTa")
        AT = [big_pool.tile([128, S], bf16, tag=f"AT{p}", name=f"AT{p}")
              for p in range(PAIRS)]
        # Vaug[k, st, h, 0:64] = V_h[k of s-tile st, :]; col 64 = ones (-> l)
        Vaug = big_pool.tile([128, NST, HPC, HD + 1], bf16, tag="Vaug",
                             name="Vaug")
        nc.vector.memset(Vaug[:, :, :, HD:HD + 1], 1.0)

        # ---- projection pieces, fed into the attention loop so ScalarE exp
        # ---- work overlaps projection PE work
        def proj_qk(w0, dstT, j):
            qsl = slice(j * QCH, (j + 1) * QCH)
            ps2 = psum_s.tile([128, 2, QCH], f32, tag="s2", name="ps2")
            for ct in range(PAIRS):
                for kt in range(8):
                    nc.tensor.matmul(
                        ps2[:, ct, :],
                        qkv_sb[:, kt, w0 + ct * 128:w0 + (ct + 1) * 128],
                        xin[j][:, kt, :],
                        start=(kt == 0), stop=(kt == 7))
            nc.vector.tensor_copy(out=dstT[:, :, qsl], in_=ps2)

        def proj_v(g):
            # V for s-tiles 2g, 2g+1 directly in [k, hd] layout
            vps = psum_s.tile([128, 2, 256], f32, tag="s2", name="vps")
            for i in range(2):
                st = 2 * g + i
                xj = xin[st // 4]
                ssl = slice((st % 4) * 128, (st % 4) * 128 + 128)
                for kt in range(8):
                    nc.tensor.matmul(
                        vps[:, i, :],
                        xj[:, kt, ssl],
                        qkv_sb[:, kt, 512:768],
                        start=(kt == 0), stop=(kt == 7))
            nc.vector.tensor_copy(
                out=Vaug[:, 2 * g:2 * g + 2, :, 0:HD],
                in_=vps.rearrange("p i (h c) -> p i h c", h=HPC))

        def round_feed(j):
            return [lambda: proj_qk(0, QTa, j),
                    lambda: proj_qk(256, KTa, j),
                    lambda: proj_v(2 * j),
                    lambda: proj_v(2 * j + 1)]

        # prologue: round 0 inputs emitted directly
        for piece in round_feed(0):
            piece()

        # ---- Phase 2: causal attention, pair-packed; then output proj ----
        def finalize_a(p, j, oAB):
            # l -> SBUF -> 1/l, pure DVE work emitted right after the pair's
            # last PV. custom-DVE ops read garbage from PSUM on real HW
            # (CoreSim accepts it), so stage the l row through SBUF first;
            # bf16 output is fine (the fp32 wrapper assert only protects the
            # bitwise seed on the input).
            lrows = small_pool.tile([1, 2, QCH], f32, tag="lrows", name="lrows")
            nc.vector.tensor_copy(out=lrows, in_=oAB[HD:HD + 1, :, :])
            linv = small_pool.tile([1, 2, QCH], bf16, tag="linv", name="linv")
            c = RECIP_CONSTS
            nc.vector._custom_dve(RECIP_OP, out=linv, in0=lrows,
                                  s0=c["s0"], s1=c["s1"], imm2=c["imm2"])
            return (p, j, oAB, linv)

        def finalize_b(st, rsb_vector=False, mul_rts=None):
            # broadcast 1/l over partitions via ones (x) linv, stage to SBUF
            # (DVE may read only ONE PSUM operand per op), then normalize
            # straight out of oAB into AT
            p, j, oAB, linv = st
            Rbr = psum_s.tile([128, 2, QCH], f32, tag="s2", name="Rbr")
            for hh in range(2):
                nc.tensor.matmul(Rbr[:, hh, :], ones_r, linv[0:1, hh, :],
                                 start=True, stop=True)
            Rsb = small_pool.tile([128, 2, QCH], f32, tag="Rsb", name="Rsb")
            if rsb_vector:
                nc.vector.tensor_copy(out=Rsb, in_=Rbr)
            else:
                nc.scalar.copy(out=Rsb, in_=Rbr)
            if mul_rts is None:
                fin_muls(st, Rsb, 0, QCH)
            else:
                mul_rts.append((st, Rsb))

        def fin_muls(st, Rsb, q0, q1):
            p, j, oAB, linv = st
            for hh in range(2):
                nc.vector.tensor_mul(
                    AT[p][hh * HD:(hh + 1) * HD, j * QCH + q0:j * QCH + q1],
                    oAB[0:HD, hh, q0:q1],
                    Rsb[hh * HD:(hh + 1) * HD, hh, q0:q1])

        def outproj_rt(j, rt, scalar_copy=False):
            # output projection for one 128-row tile of chunk j
            rsl = slice(j * QCH + rt * 128, j * QCH + (rt + 1) * 128)
            po2 = psum_s.tile([128, 2, QCH], f32, tag="s2", name="po2")
            for nch in range(DIM // QCH):
                for pp in range(PAIRS):
                    nc.tensor.matmul(
                        po2[:, nch, :], AT[pp][:, rsl],
                        ow_sb[:, pp, nch * QCH:(nch + 1) * QCH],
                        start=(pp == 0), stop=(pp == PAIRS - 1))
            ot = outst_pool.tile([128, DIM], bf16, tag="ot", name="ot")
            otv = ot.rearrange("p (a b) -> p a b", a=2)
            if scalar_copy:
                nc.scalar.copy(out=otv, in_=po2)
            else:
                nc.vector.tensor_copy(out=otv, in_=po2)
            nc.sync.dma_start(out=out[rsl, :], in_=ot)

        def outproj_pieces(j):
            return [lambda rt=rt: outproj_rt(j, rt) for rt in range(QCH // 128)]

        def tri_bc(n):
            # tri_sb broadcast over the hh axis (stride-0 free dim)
            return bass.AP(tensor=tri_sb.tensor, offset=tri_sb.offset,
                           ap=[list(tri_sb.ap[0]), [0, n]] +
                              [list(d) for d in tri_sb.ap[1:]])

        def attend_kt(p, j, kt, oAB, pv_prev):
            # S^T + exp (+ triangle mask) for (pair, ktile); PV of prev ktile
            di = kt - 4 * j
            q0 = 128 * di if di > 0 else 0
            ksl = slice(kt * 128, (kt + 1) * 128)
            sAB = psum_s.tile([128, 2, QCH], f32, tag="s2", name="sAB")
            for hh in range(2):
                nc.tensor.matmul(sAB[:, hh, q0:QCH],
                                 KTa[hh * 64:(hh + 1) * 64, p, ksl],
                                 QTa[hh * 64:(hh + 1) * 64, p,
                                     j * QCH + q0:(j + 1) * QCH],
                                 start=True, stop=True)
            pAB = pt_pool.tile([128, 2, QCH], bf16, tag="pt", name="pAB")
            nc.scalar.activation(out=pAB[:, :, q0:QCH], in_=sAB[:, :, q0:QCH],
                                 func=Exp)
            if di >= 0:
                # triangle mask on the (otherwise idle) GpSimd engine
                nc.gpsimd.tensor_mul(pAB[:, :, q0:q0 + 128],
                                     pAB[:, :, q0:q0 + 128], tri_bc(2))
            if pv_prev is not None:
                emit_pv(p, j, oAB, pv_prev, stop=False)
            return (kt, q0, pAB)

        def emit_pv(p, j, oAB, pv, stop):
            kprev, q0p, pprev = pv
            for hh in range(2):
                nc.tensor.matmul(oAB[:, hh, q0p:QCH],
                                 Vaug[:, kprev, 2 * p + hh, :],
                                 pprev[:, hh, q0p:QCH],
                                 start=(kprev == 0), stop=stop)

        pending_norm = None
        NJ = S // QCH
        feedq, ojq = [], []
        for j in range(NJ - 1):
            feedq = round_feed(j + 1)
            for p in range(PAIRS):
                nkt = 4 * (j + 1)
                oAB = psum_o.tile([HD + 1, 2, QCH], f32, tag="o", name="oAB")
                pv_prev = None
                for kt in range(nkt):
                    pv_prev = attend_kt(p, j, kt, oAB, pv_prev)
                    if feedq:
                        feedq.pop(0)()
                    elif ojq and (kt >= 2 or p == 1):
                        ojq.pop(0)()
                    if pending_norm is not None and kt == min(3, nkt - 1):
                        finalize_b(pending_norm)
                        pending_norm = None
                emit_pv(p, j, oAB, pv_prev, stop=True)
                pending_norm = finalize_a(p, j, oAB)
            while feedq:
                feedq.pop(0)()
            while ojq:
                ojq.pop(0)()
            ojq = outproj_pieces(j)

        # last chunk: interleave the two pairs per ktile so both finish
        # together and the tail (finalize + outproj + out DMA) is short
        j = NJ - 1
        nkt = 4 * NJ
        oABs = [psum_o.tile([HD + 1, 2, QCH], f32, tag="o", name=f"oL{p}")
                for p in range(PAIRS)]
        pvp = [None, None]
        for kt in range(nkt):
            for p in range(PAIRS):
                pvp[p] = attend_kt(p, j, kt, oABs[p], pvp[p])
            if pending_norm is not None and kt == 0:
                # must be emitted before oL1's first PV needs its psum_o
                # slot; linv has long been ready so the matmul doesn't block
                finalize_b(pending_norm, rsb_vector=True)
                pending_norm = None
            if ojq and kt >= 1:
                ojq.pop(0)()
        while ojq:
            ojq.pop(0)()
        # tail: finalize both pairs, then normalize + project one 128-row
        # tile at a time so PE/DVE/ACT/DMA pipeline through the epilogue
        sts = []
        mul_rts = []
        for p in range(PAIRS):
            emit_pv(p, j, oABs[p], pvp[p], stop=True)
            sts.append(finalize_a(p, j, oABs[p]))
        for p in range(PAIRS):
            finalize_b(sts[p], mul_rts=mul_rts)
        for rt in range(QCH // 128):
            for st, Rsb in mul_rts:
                fin_muls(st, Rsb, rt * 128, (rt + 1) * 128)
            outproj_rt(j, rt, scalar_copy=(rt % 2 == 1))

        if debug:
            nc.sync.dma_start(out=dbg["dQT0"], in_=QTa[:, 0, :])
            nc.sync.dma_start(out=dbg["dKT0"], in_=KTa[:, 0, :])
            nc.sync.dma_start(out=dbg["dVaug"], in_=Vaug)
            nc.sync.dma_start(out=dbg["dAT0"], in_=AT[0])

    nc.compile()
    return nc


def _get_nc():
    global _nc_cache
    if _nc_cache is None:
        _nc_cache = _build()
    return _nc_cache


def _prep_inputs(x, qw, kw, vw, ow):
    import ml_dtypes
    bf16 = ml_dtypes.bfloat16
    # undo interleaved head split: head h = cols h::16 -> contiguous blocks
    perm = np.concatenate([np.arange(h, DIM, HEADS) for h in range(HEADS)])
    qw_p = (qw[:, perm] / np.float32(np.sqrt(DIM))).astype(bf16)
    kw_p = kw[:, perm].astype(bf16)
    vw_p = vw[:, perm].astype(bf16)
    ow_p = ow[perm, :].astype(bf16)

    in_maps = []
    for c in range(N_CORES):
        b, hg = c // 4, c % 4
        csl = slice(hg * 256, (hg + 1) * 256)
        # pack so each SBUF partition is one contiguous DRAM run
        qkv = np.concatenate(
            [qw_p[:, csl], kw_p[:, csl], vw_p[:, csl]], axis=1)
        qkv = np.ascontiguousarray(qkv.reshape(8, 128, 768).transpose(1, 0, 2))
        xT = x[b].T.astype(bf16)
        xpk = np.ascontiguousarray(
            xT.reshape(8, 128, S // QCH, QCH).transpose(2, 1, 0, 3))
        owk = np.ascontiguousarray(
            ow_p[csl, :].reshape(2, 128, DIM).transpose(1, 0, 2))
        in_maps.append({"xp": xpk, "qkv": qkv, "owp": owk})
    return in_maps


def kernel(x, qw, kw, vw, ow, _trace=False):
    from concourse.bass_utils import run_bass_kernel_spmd

    if _trace:
        _install_ntff_hook()

    nc = _get_nc()
    in_maps = _prep_inputs(x, qw, kw, vw, ow)
    res = run_bass_kernel_spmd(nc, in_maps, core_ids=list(range(N_CORES)),
                               trace=_trace)
    parts = [r["out"].astype(np.float32) for r in res.results]
    outb = [parts[0] + parts[1] + parts[2] + parts[3],
            parts[4] + parts[5] + parts[6] + parts[7]]
    full = np.stack(outb).astype(np.float32)
    if _trace:
        kernel.last_results = res
        if res.exec_time_ns is not None:
            print(f"HW exec time: {res.exec_time_ns} ns")
        if res.instructions_and_trace:
            print(f"trace: {res.instructions_and_trace[1]}")
    return full


def _install_ntff_hook():
    """The image's antenv lacks axon_hooks; synthesize it so trace=True works."""
    import types
    if 'antenv.axon_hooks' in sys.modules:
        return
    mod = types.ModuleType('antenv.axon_hooks')
    mod._hook = None
    mod.set_axon_ntff_profile_hook = lambda h: setattr(mod, '_hook', h)
    mod.get_axon_ntff_profile_hook = lambda: mod._hook
    sys.modules['antenv.axon_hooks'] = mod
    import antenv
    antenv.axon_hooks = mod
    from trn_agent_boot.trn_boot import _ntff_profile_via_ctypes
    mod.set_axon_ntff_profile_hook(
        _ntff_profile_via_ctypes('/opt/axon/libaxon_pjrt.so'))
